# revision 62
# baseline (speedup 1.0000x reference)
"""Locally-connected 2D layer on 8 Trainium2 NeuronCores.

Problem: x[128,3,64,64] f32, per-position weights W[60,60,32,75], bias b[60,60,32]
  out[b,o,y,x] = sum_k patches[b,y,x,k] * W[y,x,o,k] + b[y,x,o],  k=(c,dy,dx)

Strategy (spatial sharding over output rows, 8 rows/core, memory-regime):
  - Input ring of 4 BLOCKS x 32 partitions; each block holds TWO input rows
    (15 dx-expanded patch planes + a ones plane that carries the bias, x2).
    Input row r lives at block (r//2)%4, sub-slot r%2.  A block's re-fill
    comes 3+ output rows after its last reader, so fills stream at full
    queue rate instead of serializing with the matmuls (the old mod-5 ring
    stalled ~3.6us per row on the fill chain).
  - Output row k contracts blocks (k//2 .. k//2+2)%4 = 96 partitions (the
    6th covered input row gets zero weights).  Matmul partition bases stay
    32-aligned: unwrapped windows use one matmul, wrapped ones use two
    accumulating matmuls over the low/high partition segments; the host
    compact W layout concatenates active blocks in ascending order.
  - All tensors bf16 on the wire (PSUM f32; host widens output).
  - Queue discipline: hardware DGE queues (sync + scalar) carry x fills,
    W rows 0-3, and the stores (split across both); W rows 4-7 prefetch on
    the gpsimd software queue.  Packets are shaped to >=7680B.
  - Per output row: 5 column chunks x 12 matmul-groups -> psum[32j:32j+32,
    128], DVE copy psum->bf16 pair tile, one 983KB store per row pair.
"""

import numpy as np

B, C, H, WIDTH = 128, 3, 64, 64
KH = KW = 5
RY = RX = 60
O = 32
K = 75
NCORES = 8
RPC = 8             # output rows computed per core (8*8=64, last 4 dropped)
INR = RPC + KH - 1  # 12 input rows per core
PADH = NCORES * RPC + KH - 1  # 68
NG = 15             # groups of 4 x-positions per row
NCH = 5             # matmul column chunks per row (3 groups each)
GPC = 3             # groups per chunk
FXB = RX * B        # 7680 elements per patch plane
NPL = KH * C        # 15 patch planes per input row
SUB = 16            # partitions per input row (15 planes + ones)
BLK = 32            # partitions per block (2 input rows)
NBLK = 4
K2 = 3 * BLK        # contraction depth per output row (96)
WROW = RX * O       # 1920 weight elems per (row, partition)
NWP = 4             # W pieces (2 rows each)

_cache = {}


def _build():
    import concourse.bass as bass
    import concourse.bacc as bacc
    import concourse.tile as tile
    import concourse.mybir as mybir

    f32 = mybir.dt.float32
    bf16 = mybir.dt.bfloat16
    nc = bacc.Bacc("TRN2", target_bir_lowering=False, debug=False,
                   num_devices=NCORES)
    xpr_d = nc.dram_tensor("xpr", [INR, NPL, FXB], bf16, kind="ExternalInput")
    wh_d = nc.dram_tensor("wh", [RPC, 5 * SUB, WROW], bf16,
                          kind="ExternalInput")
    ones_d = nc.dram_tensor("ones", [NBLK * 2, FXB], bf16,
                            kind="ExternalInput")
    oc_d = nc.dram_tensor("oc", [RPC // 2, 128, 2 * NG * B], bf16,
                          kind="ExternalOutput")

    HFB = FXB // 2  # column half (3840 elems -> 7680B packets)
    QFB = FXB // 4  # column quarter (1920 elems)

    with tile.TileContext(nc) as tc:
        with (
            tc.tile_pool(name="const", bufs=1) as cpool,
            tc.tile_pool(name="w", bufs=8) as wpool,
            tc.tile_pool(name="os", bufs=3) as opool,
            tc.tile_pool(name="ps", bufs=6, space=bass.MemorySpace.PSUM) as ppool,
        ):
            xp = cpool.tile([NBLK * BLK, FXB], bf16)  # [128, 7680]

            wts = {}
            # per-row aligned memset covers for the read-but-unloaded
            # partitions (DVE requires 32-aligned bases / 0-64 for 64-spans)
            ZCOV = {0: [(64, 96)], 1: [(0, 32)], 2: [(0, 32), (96, 128)],
                    3: [(0, 64)], 4: [(0, 64)], 5: [(32, 64), (64, 96)],
                    6: [(32, 64), (64, 96)], 7: [(64, 128)]}

            def load_w(k, eng, eng2=None):
                # slot-absolute W tile: zero the inactive window partitions,
                # then DMA the 5 active sub-slots from the compact DRAM row
                wt = wpool.tile([NBLK * BLK, WROW], bf16, name=f"wp{k}",
                                tag=f"wp{k}", bufs=1)
                for z0, z1 in ZCOV[k]:
                    nc.vector.memset(wt[z0:z1, :], 0.0)
                if k <= 3:  # one contiguous run [16k : 16k+80]
                    eng.dma_start(wt[SUB * k:SUB * k + 5 * SUB, :], wh_d[k])
                else:  # wrap: dst [0 : 16(k-3)] and [16k : 128]
                    nlo = SUB * (k - 3)
                    eng.dma_start(wt[0:nlo, :], wh_d[k][0:nlo])
                    (eng2 or eng).dma_start(wt[SUB * k:128, :],
                                            wh_d[k][nlo:5 * SUB])
                wts[k] = wt

            def fill(r, f0, f1, eng):
                # input row r -> block (r//2)%4, sub-slot r%2
                p0 = BLK * ((r // 2) % NBLK) + SUB * (r % 2)
                eng.dma_start(xp[p0:p0 + NPL, f0:f1], xpr_d[r, :, f0:f1])

            # ones planes (bias carriers), one per sub-slot, as a single
            # strided-partition DMA (engine issue time is ~1us per DMA
            # instruction, so consolidating 8 tiny DMAs matters)
            nc.sync.dma_start(xp[NPL:128:SUB, :], ones_d[:])
            # MINIMAL prologue: the first matmul waits for every DMA issued
            # before it (compute-phase barrier), so only row 0's true needs
            # go here: ones planes, W row 0, input rows 0-5.  Everything
            # else is issued inside the row loop, after compute starts.
            load_w(0, nc.sync, nc.scalar)
            for r in range(6):  # full-row fills: fewer engine issues
                fill(r, 0, FXB, nc.scalar if r % 2 == 0 else nc.sync)

            for k in range(RPC):
                wrow = wts[k]  # [128, 1920], slot-absolute
                m4 = (k // 2) % NBLK
                # W is slot-absolute with zeros on inactive sub-slots, so a
                # full-depth contraction is exact and keeps the PE tile
                # position fixed (pipelined).  m4=0 rows use the legal
                # narrow [0:96) span, which avoids false RAW deps on the
                # block-0/1 refills; other bases would violate the PE's
                # partition-alignment rules, so they contract all 128.
                segs = [(0, 96)] if m4 == 0 else [(0, 128)]
                if k % 2 == 0:
                    ot = opool.tile([128, 2 * NG * B], bf16)  # [128, 3840]
                ob = (k % 2) * NG * B
                for ci in range(NCH):
                    # full 2KB PSUM bank; only the first GPC*B cols are used
                    pt = ppool.tile([128, 4 * B], f32)
                    for gs in range(GPC):
                        for j in range(4):
                            xpos = (ci * GPC + gs) * 4 + j
                            po = pt[32 * j:32 * (j + 1),
                                    gs * B:(gs + 1) * B]
                            wcol = slice(xpos * O, (xpos + 1) * O)
                            xcol = slice(xpos * B, (xpos + 1) * B)
                            for si, (x0, x1) in enumerate(segs):
                                nc.tensor.matmul(
                                    po, wrow[x0:x1, wcol],
                                    xp[x0:x1, xcol],
                                    start=(si == 0),
                                    stop=(si == len(segs) - 1),
                                    tile_position=(x0, 32 * j),
                                )
                    nc.vector.tensor_copy(
                        ot[:, ob + ci * GPC * B:ob + (ci + 1) * GPC * B],
                        pt[:, :GPC * B])
                    if k == 0 and ci == 0:
                        # deferred loads, issued after row 0's first chunk
                        # so they sit past the compute-phase barrier: W row
                        # 1, input rows 6-7, SW-queue W prefetches
                        load_w(1, nc.scalar, nc.sync)
                        fill(6, 0, FXB, nc.sync)
                        fill(7, 0, FXB, nc.scalar)
                        for pc in range(2, 8):
                            load_w(pc, nc.gpsimd)
                    if k in (1, 3) and ci >= 1:
                        # re-fill block (k-1)/2 with input rows k+7, k+8 in
                        # column quarters, each gated on the last chunk of
                        # this row that read those columns
                        f0, f1 = (ci - 1) * QFB, ci * QFB
                        fill(k + 7, f0, f1,
                             nc.sync if ci % 2 == 1 else nc.scalar)
                        fill(k + 8, f0, f1,
                             nc.scalar if ci % 2 == 1 else nc.sync)
                if k in (1, 3, 5):
                    nc.sync.dma_start(oc_d[k // 2, 0:64], ot[0:64, :])
                    nc.scalar.dma_start(oc_d[k // 2, 64:128], ot[64:128, :])
                elif k >= 6:
                    # last pair: store each row as soon as it is done, and
                    # split across both queues, to shrink the drain tail
                    c0, c1 = (k % 2) * NG * B, (k % 2 + 1) * NG * B
                    nc.sync.dma_start(oc_d[3, 0:64, c0:c1],
                                      ot[0:64, c0:c1])
                    nc.scalar.dma_start(oc_d[3, 64:128, c0:c1],
                                        ot[64:128, c0:c1])

    nc.compile()
    return nc


def _get_nc():
    if "nc" not in _cache:
        _cache["nc"] = _build()
    return _cache["nc"]


def _prep_inputs(x, W, b):
    import ml_dtypes
    bf = ml_dtypes.bfloat16
    x = np.asarray(x, np.float32)
    W = np.asarray(W, np.float32)
    b = np.asarray(b, np.float32)
    xh = np.zeros((PADH, C, WIDTH, B), np.float32)
    xh[:H] = x.transpose(2, 1, 3, 0)  # [row, c, w, batch]
    # patch planes: xpr_full[r, c*KW+dx, x, b] = xh[r, c, x+dx, b]
    xpr_full = np.zeros((PADH, C, KW, RX, B), np.float32)
    for dx in range(KW):
        xpr_full[:, :, dx] = xh[:, :, dx:dx + RX]
    xpr_full = xpr_full.reshape(PADH, C * KW, FXB)
    Wfull = W.transpose(0, 3, 1, 2)  # [RY, K, RX, O]
    in_maps = []
    for i in range(NCORES):
        # compact W: per row, the 5 active sub-slots (15 weight planes +
        # bias/zero plane each) ordered by ascending absolute partition
        whc = np.zeros((RPC, 5 * SUB, RX, O), np.float32)
        for k in range(RPC):
            y = RPC * i + k
            if y < RY:
                w5 = Wfull[y].reshape(C, KH, KW, RX, O)
                rows = sorted(range(k, k + 5), key=lambda rr: rr % 8)
                for i2, rr in enumerate(rows):
                    dy = rr - k
                    whc[k, i2 * SUB:i2 * SUB + NPL] = \
                        w5[:, dy].reshape(NPL, RX, O)
                    if dy == 0:
                        whc[k, i2 * SUB + NPL] = b[y]
        wres = np.ascontiguousarray(whc.reshape(RPC, 5 * SUB, WROW))
        in_maps.append({
            "xpr": np.ascontiguousarray(
                xpr_full[RPC * i:RPC * i + INR]).astype(bf),
            "wh": wres.astype(bf),
            "ones": np.ones((NBLK * 2, FXB), bf),
        })
    return in_maps


def kernel(x, W, b):
    from concourse.bass_utils import run_bass_kernel_spmd

    nc = _get_nc()
    in_maps = _prep_inputs(x, W, b)
    br = run_bass_kernel_spmd(nc, in_maps, list(range(NCORES)),
                              **_cache.get("run_kwargs", {}))
    _cache["last_run"] = br
    oc = np.stack([np.asarray(br.results[i]["oc"]).astype(np.float32)
                   for i in range(NCORES)])
    # oc: [core*pair, (j,o), (k2, g, b)]
    oc = oc.reshape(NCORES * RPC // 2, 4, O, 2, NG, B)
    # out[b, o, row=pair*2+k2, x=g*4+j]
    oc = oc.transpose(5, 2, 0, 3, 4, 1)  # [b, o, pair, k2, g, j]
    out = oc.reshape(B, O, NCORES * RPC // 2 * 2, NG * 4)
    return np.ascontiguousarray(out[:, :, :RY, :])


# revision 65
# speedup vs baseline: 1.0077x; 1.0077x over previous
"""Locally-connected 2D layer on 8 Trainium2 NeuronCores.

Problem: x[128,3,64,64] f32, per-position weights W[60,60,32,75], bias b[60,60,32]
  out[b,o,y,x] = sum_k patches[b,y,x,k] * W[y,x,o,k] + b[y,x,o],  k=(c,dy,dx)

Strategy (spatial sharding over output rows, 8 rows/core, memory-regime):
  - Input ring of 4 BLOCKS x 32 partitions; each block holds TWO input rows
    (15 dx-expanded patch planes + a ones plane that carries the bias, x2).
    Input row r lives at block (r//2)%4, sub-slot r%2.  A block's re-fill
    comes 3+ output rows after its last reader, so fills stream at full
    queue rate instead of serializing with the matmuls (the old mod-5 ring
    stalled ~3.6us per row on the fill chain).
  - Output row k contracts blocks (k//2 .. k//2+2)%4 = 96 partitions (the
    6th covered input row gets zero weights).  Matmul partition bases stay
    32-aligned: unwrapped windows use one matmul, wrapped ones use two
    accumulating matmuls over the low/high partition segments; the host
    compact W layout concatenates active blocks in ascending order.
  - All tensors bf16 on the wire (PSUM f32; host widens output).
  - Queue discipline: hardware DGE queues (sync + scalar) carry x fills,
    W rows 0-3, and the stores (split across both); W rows 4-7 prefetch on
    the gpsimd software queue.  Packets are shaped to >=7680B.
  - Per output row: 5 column chunks x 12 matmul-groups -> psum[32j:32j+32,
    128], DVE copy psum->bf16 pair tile, one 983KB store per row pair.
"""

import numpy as np

B, C, H, WIDTH = 128, 3, 64, 64
KH = KW = 5
RY = RX = 60
O = 32
K = 75
NCORES = 8
RPC = 8             # output rows computed per core (8*8=64, last 4 dropped)
INR = RPC + KH - 1  # 12 input rows per core
PADH = NCORES * RPC + KH - 1  # 68
NG = 15             # groups of 4 x-positions per row
NCH = 5             # matmul column chunks per row (3 groups each)
GPC = 3             # groups per chunk
FXB = RX * B        # 7680 elements per patch plane
NPL = KH * C        # 15 patch planes per input row
SUB = 16            # partitions per input row (15 planes + ones)
BLK = 32            # partitions per block (2 input rows)
NBLK = 4
K2 = 3 * BLK        # contraction depth per output row (96)
WROW = RX * O       # 1920 weight elems per (row, partition)
NWP = 4             # W pieces (2 rows each)

_cache = {}


def _build():
    import concourse.bass as bass
    import concourse.bacc as bacc
    import concourse.tile as tile
    import concourse.mybir as mybir

    f32 = mybir.dt.float32
    bf16 = mybir.dt.bfloat16
    nc = bacc.Bacc("TRN2", target_bir_lowering=False, debug=False,
                   num_devices=NCORES)
    xpr_d = nc.dram_tensor("xpr", [INR, NPL, FXB], bf16, kind="ExternalInput")
    wh_d = nc.dram_tensor("wh", [RPC, 5 * SUB, WROW], bf16,
                          kind="ExternalInput")
    ones_d = nc.dram_tensor("ones", [NBLK * 2, FXB], bf16,
                            kind="ExternalInput")
    oc_d = nc.dram_tensor("oc", [RPC // 2, 128, 2 * NG * B], bf16,
                          kind="ExternalOutput")

    HFB = FXB // 2  # column half (3840 elems -> 7680B packets)
    QFB = FXB // 4  # column quarter (1920 elems)

    with tile.TileContext(nc) as tc:
        with (
            tc.tile_pool(name="const", bufs=1) as cpool,
            tc.tile_pool(name="w", bufs=8) as wpool,
            tc.tile_pool(name="os", bufs=3) as opool,
            tc.tile_pool(name="ps", bufs=6, space=bass.MemorySpace.PSUM) as ppool,
        ):
            xp = cpool.tile([NBLK * BLK, FXB], bf16)  # [128, 7680]

            wts = {}
            # per-row aligned memset covers for the read-but-unloaded
            # partitions (DVE requires 32-aligned bases / 0-64 for 64-spans)
            ZCOV = {0: [(64, 96)], 1: [(0, 32)], 2: [(0, 32), (96, 128)],
                    3: [(0, 64)], 4: [(0, 64)], 5: [(32, 64), (64, 96)],
                    6: [(32, 64), (64, 96)], 7: [(64, 128)]}

            def load_w(k, eng, eng2=None):
                # slot-absolute W tile: zero the inactive window partitions,
                # then DMA the 5 active sub-slots from the compact DRAM row
                # (per-queue column halves when eng2 is given)
                wt = wpool.tile([NBLK * BLK, WROW], bf16, name=f"wp{k}",
                                tag=f"wp{k}", bufs=1)
                for z0, z1 in ZCOV[k]:
                    nc.vector.memset(wt[z0:z1, :], 0.0)
                if k <= 3:
                    runs = [(SUB * k, SUB * k + 5 * SUB, 0)]
                else:
                    nlo = SUB * (k - 3)
                    runs = [(0, nlo, 0), (SUB * k, 128, nlo)]
                for p0, p1, s0 in runs:
                    n = p1 - p0
                    if eng2 is None:
                        eng.dma_start(wt[p0:p1, :], wh_d[k][s0:s0 + n])
                    else:
                        eng.dma_start(wt[p0:p1, 0:WROW // 2],
                                      wh_d[k][s0:s0 + n, 0:WROW // 2])
                        eng2.dma_start(wt[p0:p1, WROW // 2:WROW],
                                       wh_d[k][s0:s0 + n, WROW // 2:WROW])
                wts[k] = wt

            def fill(r, f0, f1, eng):
                # input row r -> block (r//2)%4, sub-slot r%2
                p0 = BLK * ((r // 2) % NBLK) + SUB * (r % 2)
                eng.dma_start(xp[p0:p0 + NPL, f0:f1], xpr_d[r, :, f0:f1])

            # ones planes (bias carriers), one per sub-slot, as a single
            # strided-partition DMA (engine issue time is ~1us per DMA
            # instruction, so consolidating 8 tiny DMAs matters)
            nc.sync.dma_start(xp[NPL:128:SUB, :], ones_d[:])
            # MINIMAL prologue: the first matmul waits for every DMA issued
            # before it (compute-phase barrier), so only row 0's true needs
            # go here: ones planes, W row 0, input rows 0-5.  Everything
            # else is issued inside the row loop, after compute starts.
            load_w(0, nc.sync, nc.scalar)
            for r in range(6):  # full-row fills: fewer engine issues
                fill(r, 0, FXB, nc.scalar if r % 2 == 0 else nc.sync)

            for k in range(RPC):
                wrow = wts[k]  # [128, 1920], slot-absolute
                m4 = (k // 2) % NBLK
                # W is slot-absolute with zeros on inactive sub-slots, so a
                # full-depth contraction is exact and keeps the PE tile
                # position fixed (pipelined).  m4=0 rows use the legal
                # narrow [0:96) span, which avoids false RAW deps on the
                # block-0/1 refills; other bases would violate the PE's
                # partition-alignment rules, so they contract all 128.
                segs = [(0, 96)] if m4 == 0 else [(0, 128)]
                if k % 2 == 0:
                    ot = opool.tile([128, 2 * NG * B], bf16)  # [128, 3840]
                ob = (k % 2) * NG * B
                for ci in range(NCH):
                    # full 2KB PSUM bank; only the first GPC*B cols are used
                    pt = ppool.tile([128, 4 * B], f32)
                    for gs in range(GPC):
                        for j in range(4):
                            xpos = (ci * GPC + gs) * 4 + j
                            po = pt[32 * j:32 * (j + 1),
                                    gs * B:(gs + 1) * B]
                            wcol = slice(xpos * O, (xpos + 1) * O)
                            xcol = slice(xpos * B, (xpos + 1) * B)
                            for si, (x0, x1) in enumerate(segs):
                                nc.tensor.matmul(
                                    po, wrow[x0:x1, wcol],
                                    xp[x0:x1, xcol],
                                    start=(si == 0),
                                    stop=(si == len(segs) - 1),
                                    tile_position=(x0, 32 * j),
                                )
                    nc.vector.tensor_copy(
                        ot[:, ob + ci * GPC * B:ob + (ci + 1) * GPC * B],
                        pt[:, :GPC * B])
                    if k == 0 and ci == 0:
                        # deferred loads in strict need-order (every DMA
                        # issued here gates all later compute phases)
                        load_w(1, nc.scalar, nc.sync)
                        fill(6, 0, FXB, nc.sync)
                        fill(7, 0, FXB, nc.scalar)
                    elif k == 0 and ci == 2:
                        load_w(2, nc.sync, nc.scalar)
                    elif k == 0 and ci == 4:
                        load_w(3, nc.scalar, nc.sync)
                    if k in (1, 3) and ci >= 1:
                        # re-fill block (k-1)/2 with input rows k+7, k+8 in
                        # column quarters, each gated on the last chunk of
                        # this row that read those columns
                        f0, f1 = (ci - 1) * QFB, ci * QFB
                        fill(k + 7, f0, f1,
                             nc.sync if ci % 2 == 1 else nc.scalar)
                        fill(k + 8, f0, f1,
                             nc.scalar if ci % 2 == 1 else nc.sync)
                if k == 1:
                    load_w(4, nc.sync, nc.scalar)
                    load_w(5, nc.scalar, nc.sync)
                elif k == 3:
                    load_w(6, nc.sync, nc.scalar)
                    load_w(7, nc.scalar, nc.sync)
                if k in (1, 5):
                    nc.sync.dma_start(oc_d[k // 2, 0:64], ot[0:64, :])
                    nc.scalar.dma_start(oc_d[k // 2, 64:128], ot[64:128, :])
                elif k == 3:
                    # pair 1 store rides the software queue: it gates only
                    # the end barrier, freeing HW-queue time for loads
                    nc.gpsimd.dma_start(oc_d[1], ot[:])
                elif k >= 6:
                    # last pair: store each row as soon as it is done, and
                    # split across both queues, to shrink the drain tail
                    c0, c1 = (k % 2) * NG * B, (k % 2 + 1) * NG * B
                    nc.sync.dma_start(oc_d[3, 0:64, c0:c1],
                                      ot[0:64, c0:c1])
                    nc.scalar.dma_start(oc_d[3, 64:128, c0:c1],
                                        ot[64:128, c0:c1])

    nc.compile()
    return nc


def _get_nc():
    if "nc" not in _cache:
        _cache["nc"] = _build()
    return _cache["nc"]


def _prep_inputs(x, W, b):
    import ml_dtypes
    bf = ml_dtypes.bfloat16
    x = np.asarray(x, np.float32)
    W = np.asarray(W, np.float32)
    b = np.asarray(b, np.float32)
    xh = np.zeros((PADH, C, WIDTH, B), np.float32)
    xh[:H] = x.transpose(2, 1, 3, 0)  # [row, c, w, batch]
    # patch planes: xpr_full[r, c*KW+dx, x, b] = xh[r, c, x+dx, b]
    xpr_full = np.zeros((PADH, C, KW, RX, B), np.float32)
    for dx in range(KW):
        xpr_full[:, :, dx] = xh[:, :, dx:dx + RX]
    xpr_full = xpr_full.reshape(PADH, C * KW, FXB)
    Wfull = W.transpose(0, 3, 1, 2)  # [RY, K, RX, O]
    in_maps = []
    for i in range(NCORES):
        # compact W: per row, the 5 active sub-slots (15 weight planes +
        # bias/zero plane each) ordered by ascending absolute partition
        whc = np.zeros((RPC, 5 * SUB, RX, O), np.float32)
        for k in range(RPC):
            y = RPC * i + k
            if y < RY:
                w5 = Wfull[y].reshape(C, KH, KW, RX, O)
                rows = sorted(range(k, k + 5), key=lambda rr: rr % 8)
                for i2, rr in enumerate(rows):
                    dy = rr - k
                    whc[k, i2 * SUB:i2 * SUB + NPL] = \
                        w5[:, dy].reshape(NPL, RX, O)
                    if dy == 0:
                        whc[k, i2 * SUB + NPL] = b[y]
        wres = np.ascontiguousarray(whc.reshape(RPC, 5 * SUB, WROW))
        in_maps.append({
            "xpr": np.ascontiguousarray(
                xpr_full[RPC * i:RPC * i + INR]).astype(bf),
            "wh": wres.astype(bf),
            "ones": np.ones((NBLK * 2, FXB), bf),
        })
    return in_maps


def kernel(x, W, b):
    from concourse.bass_utils import run_bass_kernel_spmd

    nc = _get_nc()
    in_maps = _prep_inputs(x, W, b)
    br = run_bass_kernel_spmd(nc, in_maps, list(range(NCORES)),
                              **_cache.get("run_kwargs", {}))
    _cache["last_run"] = br
    oc = np.stack([np.asarray(br.results[i]["oc"]).astype(np.float32)
                   for i in range(NCORES)])
    # oc: [core*pair, (j,o), (k2, g, b)]
    oc = oc.reshape(NCORES * RPC // 2, 4, O, 2, NG, B)
    # out[b, o, row=pair*2+k2, x=g*4+j]
    oc = oc.transpose(5, 2, 0, 3, 4, 1)  # [b, o, pair, k2, g, j]
    out = oc.reshape(B, O, NCORES * RPC // 2 * 2, NG * 4)
    return np.ascontiguousarray(out[:, :, :RY, :])


# revision 70
# speedup vs baseline: 1.0222x; 1.0143x over previous
"""Locally-connected 2D layer on 8 Trainium2 NeuronCores.

Problem: x[128,3,64,64] f32, per-position weights W[60,60,32,75], bias b[60,60,32]
  out[b,o,y,x] = sum_k patches[b,y,x,k] * W[y,x,o,k] + b[y,x,o],  k=(c,dy,dx)

Strategy (spatial sharding over output rows, 8 rows/core, memory-regime):
  - Input ring of 4 BLOCKS x 32 partitions; each block holds TWO input rows
    (15 dx-expanded patch planes + a ones plane that carries the bias, x2).
    Input row r lives at block (r//2)%4, sub-slot r%2.  A block's re-fill
    comes 3+ output rows after its last reader, so fills stream at full
    queue rate instead of serializing with the matmuls (the old mod-5 ring
    stalled ~3.6us per row on the fill chain).
  - Output row k contracts blocks (k//2 .. k//2+2)%4 = 96 partitions (the
    6th covered input row gets zero weights).  Matmul partition bases stay
    32-aligned: unwrapped windows use one matmul, wrapped ones use two
    accumulating matmuls over the low/high partition segments; the host
    compact W layout concatenates active blocks in ascending order.
  - All tensors bf16 on the wire (PSUM f32; host widens output).
  - Queue discipline: hardware DGE queues (sync + scalar) carry x fills,
    W rows 0-3, and the stores (split across both); W rows 4-7 prefetch on
    the gpsimd software queue.  Packets are shaped to >=7680B.
  - Per output row: 5 column chunks x 12 matmul-groups -> psum[32j:32j+32,
    128], DVE copy psum->bf16 pair tile, one 983KB store per row pair.
"""

import numpy as np

B, C, H, WIDTH = 128, 3, 64, 64
KH = KW = 5
RY = RX = 60
O = 32
K = 75
NCORES = 8
RPC = 8             # output rows computed per core (8*8=64, last 4 dropped)
INR = RPC + KH - 1  # 12 input rows per core
PADH = NCORES * RPC + KH - 1  # 68
NG = 15             # groups of 4 x-positions per row
NCH = 5             # matmul column chunks per row (3 groups each)
GPC = 3             # groups per chunk
FXB = RX * B        # 7680 elements per patch plane
NPL = KH * C        # 15 patch planes per input row
SUB = 16            # partitions per input row (15 planes + ones)
BLK = 32            # partitions per block (2 input rows)
NBLK = 4
K2 = 3 * BLK        # contraction depth per output row (96)
WROW = RX * O       # 1920 weight elems per (row, partition)
NWP = 4             # W pieces (2 rows each)

_cache = {}


def _build():
    import concourse.bass as bass
    import concourse.bacc as bacc
    import concourse.tile as tile
    import concourse.mybir as mybir

    f32 = mybir.dt.float32
    bf16 = mybir.dt.bfloat16
    nc = bacc.Bacc("TRN2", target_bir_lowering=False, debug=False,
                   num_devices=NCORES)
    xpr_d = nc.dram_tensor("xpr", [INR, NPL, FXB], bf16, kind="ExternalInput")
    wh_d = nc.dram_tensor("wh", [RPC, 5 * SUB, WROW], bf16,
                          kind="ExternalInput")
    ones_d = nc.dram_tensor("ones", [NBLK * 2, FXB], bf16,
                            kind="ExternalInput")
    oc_d = nc.dram_tensor("oc", [RPC // 2, 128, 2 * NG * B], bf16,
                          kind="ExternalOutput")

    HFB = FXB // 2  # column half (3840 elems -> 7680B packets)
    QFB = FXB // 4  # column quarter (1920 elems)

    with tile.TileContext(nc) as tc:
        with (
            tc.tile_pool(name="const", bufs=1) as cpool,
            tc.tile_pool(name="w", bufs=8) as wpool,
            tc.tile_pool(name="os", bufs=3) as opool,
            tc.tile_pool(name="ps", bufs=6, space=bass.MemorySpace.PSUM) as ppool,
        ):
            xp = cpool.tile([NBLK * BLK, FXB], bf16)  # [128, 7680]

            wts = {}
            # per-row aligned memset covers for the read-but-unloaded
            # partitions (DVE requires 32-aligned bases / 0-64 for 64-spans)
            ZCOV = {0: [(64, 96)], 1: [(0, 32)], 2: [(0, 32), (96, 128)],
                    3: [(0, 64)], 4: [(0, 64)], 5: [(32, 64), (64, 96)],
                    6: [(32, 64), (64, 96)], 7: [(64, 128)]}

            def load_w(k, eng, eng2=None):
                # slot-absolute W tile: zero the inactive window partitions,
                # then DMA the 5 active sub-slots from the compact DRAM row
                # (per-queue column halves when eng2 is given)
                wt = wpool.tile([NBLK * BLK, WROW], bf16, name=f"wp{k}",
                                tag=f"wp{k}", bufs=1)
                for z0, z1 in ZCOV[k]:
                    nc.vector.memset(wt[z0:z1, :], 0.0)
                if k <= 3:
                    # one contiguous run; column halves across two queues
                    p0 = SUB * k
                    if eng2 is None:
                        eng.dma_start(wt[p0:p0 + 5 * SUB, :], wh_d[k])
                    else:
                        eng.dma_start(wt[p0:p0 + 5 * SUB, 0:WROW // 2],
                                      wh_d[k][:, 0:WROW // 2])
                        eng2.dma_start(wt[p0:p0 + 5 * SUB, WROW // 2:WROW],
                                       wh_d[k][:, WROW // 2:WROW])
                else:
                    # wrap: low run on eng, high run on eng2
                    nlo = SUB * (k - 3)
                    eng.dma_start(wt[0:nlo, :], wh_d[k][0:nlo])
                    (eng2 or eng).dma_start(wt[SUB * k:128, :],
                                            wh_d[k][nlo:5 * SUB])
                wts[k] = wt

            def fill(r, f0, f1, eng):
                # input row r -> block (r//2)%4, sub-slot r%2
                p0 = BLK * ((r // 2) % NBLK) + SUB * (r % 2)
                eng.dma_start(xp[p0:p0 + NPL, f0:f1], xpr_d[r, :, f0:f1])

            # ones planes (bias carriers), one per sub-slot; they gate row
            # 0, so keep them off the slow software queue
            for t in range(NBLK):
                for s2 in range(2):
                    p = BLK * t + SUB * s2 + NPL
                    eng = nc.sync if (t + s2) % 2 == 0 else nc.scalar
                    eng.dma_start(xp[p:p + 1, :], ones_d[0:1])
            # MINIMAL prologue: the first matmul waits for every DMA issued
            # before it (compute-phase barrier), so only row 0's true needs
            # go here: ones planes, W row 0, input rows 0-5.  Everything
            # else is issued inside the row loop, after compute starts.
            load_w(0, nc.sync, nc.scalar)
            for r in range(6):
                fill(r, 0, HFB, nc.sync if r % 2 == 0 else nc.scalar)
                fill(r, HFB, FXB, nc.scalar if r % 2 == 0 else nc.sync)

            for k in range(RPC):
                wrow = wts[k]  # [128, 1920], slot-absolute
                m4 = (k // 2) % NBLK
                # W is slot-absolute with zeros on inactive sub-slots, so a
                # full-depth contraction is exact and keeps the PE tile
                # position fixed (pipelined).  m4=0 rows use the legal
                # narrow [0:96) span, which avoids false RAW deps on the
                # block-0/1 refills; other bases would violate the PE's
                # partition-alignment rules, so they contract all 128.
                segs = [(0, 96)] if m4 == 0 else [(0, 128)]
                if k % 2 == 0:
                    ot = opool.tile([128, 2 * NG * B], bf16)  # [128, 3840]
                ob = (k % 2) * NG * B
                for ci in range(NCH):
                    # full 2KB PSUM bank; only the first GPC*B cols are used
                    pt = ppool.tile([128, 4 * B], f32)
                    for gs in range(GPC):
                        for j in range(4):
                            xpos = (ci * GPC + gs) * 4 + j
                            po = pt[32 * j:32 * (j + 1),
                                    gs * B:(gs + 1) * B]
                            wcol = slice(xpos * O, (xpos + 1) * O)
                            xcol = slice(xpos * B, (xpos + 1) * B)
                            for si, (x0, x1) in enumerate(segs):
                                nc.tensor.matmul(
                                    po, wrow[x0:x1, wcol],
                                    xp[x0:x1, xcol],
                                    start=(si == 0),
                                    stop=(si == len(segs) - 1),
                                    tile_position=(x0, 32 * j),
                                )
                    nc.vector.tensor_copy(
                        ot[:, ob + ci * GPC * B:ob + (ci + 1) * GPC * B],
                        pt[:, :GPC * B])
                    if k == 0 and ci == 0:
                        # deferred loads, issued after row 0's first chunk
                        # so they sit past the compute-phase barrier: W row
                        # 1, input rows 6-7, SW-queue W prefetches
                        load_w(1, nc.scalar, nc.sync)
                        fill(6, 0, HFB, nc.sync)
                        fill(6, HFB, FXB, nc.scalar)
                        fill(7, 0, HFB, nc.scalar)
                        fill(7, HFB, FXB, nc.sync)
                        load_w(2, nc.gpsimd)
                        load_w(3, nc.gpsimd)
                        load_w(4, nc.gpsimd)
                    if k in (1, 3) and ci >= 1:
                        # re-fill block (k-1)/2 with input rows k+7, k+8 in
                        # column quarters, each gated on the last chunk of
                        # this row that read those columns
                        f0, f1 = (ci - 1) * QFB, ci * QFB
                        fill(k + 7, f0, f1,
                             nc.sync if ci % 2 == 1 else nc.scalar)
                        fill(k + 8, f0, f1,
                             nc.scalar if ci % 2 == 1 else nc.sync)
                if k == 1:
                    load_w(5, nc.sync, nc.scalar)
                elif k == 3:
                    load_w(6, nc.scalar, nc.sync)
                    load_w(7, nc.sync, nc.scalar)
                if k in (1, 3, 5):
                    nc.sync.dma_start(oc_d[k // 2, 0:64], ot[0:64, :])
                    nc.scalar.dma_start(oc_d[k // 2, 64:128], ot[64:128, :])
                elif k >= 6:
                    # last pair: store each row as soon as it is done, and
                    # split across both queues, to shrink the drain tail
                    c0, c1 = (k % 2) * NG * B, (k % 2 + 1) * NG * B
                    nc.sync.dma_start(oc_d[3, 0:64, c0:c1],
                                      ot[0:64, c0:c1])
                    nc.scalar.dma_start(oc_d[3, 64:128, c0:c1],
                                        ot[64:128, c0:c1])

    nc.compile()
    return nc


def _get_nc():
    if "nc" not in _cache:
        _cache["nc"] = _build()
    return _cache["nc"]


def _prep_inputs(x, W, b):
    import ml_dtypes
    bf = ml_dtypes.bfloat16
    x = np.asarray(x, np.float32)
    W = np.asarray(W, np.float32)
    b = np.asarray(b, np.float32)
    xh = np.zeros((PADH, C, WIDTH, B), np.float32)
    xh[:H] = x.transpose(2, 1, 3, 0)  # [row, c, w, batch]
    # patch planes: xpr_full[r, c*KW+dx, x, b] = xh[r, c, x+dx, b]
    xpr_full = np.zeros((PADH, C, KW, RX, B), np.float32)
    for dx in range(KW):
        xpr_full[:, :, dx] = xh[:, :, dx:dx + RX]
    xpr_full = xpr_full.reshape(PADH, C * KW, FXB)
    Wfull = W.transpose(0, 3, 1, 2)  # [RY, K, RX, O]
    in_maps = []
    for i in range(NCORES):
        # compact W: per row, the 5 active sub-slots (15 weight planes +
        # bias/zero plane each) ordered by ascending absolute partition
        whc = np.zeros((RPC, 5 * SUB, RX, O), np.float32)
        for k in range(RPC):
            y = RPC * i + k
            if y < RY:
                w5 = Wfull[y].reshape(C, KH, KW, RX, O)
                rows = sorted(range(k, k + 5), key=lambda rr: rr % 8)
                for i2, rr in enumerate(rows):
                    dy = rr - k
                    whc[k, i2 * SUB:i2 * SUB + NPL] = \
                        w5[:, dy].reshape(NPL, RX, O)
                    if dy == 0:
                        whc[k, i2 * SUB + NPL] = b[y]
        wres = np.ascontiguousarray(whc.reshape(RPC, 5 * SUB, WROW))
        in_maps.append({
            "xpr": np.ascontiguousarray(
                xpr_full[RPC * i:RPC * i + INR]).astype(bf),
            "wh": wres.astype(bf),
            "ones": np.ones((NBLK * 2, FXB), bf),
        })
    return in_maps


def kernel(x, W, b):
    from concourse.bass_utils import run_bass_kernel_spmd

    nc = _get_nc()
    in_maps = _prep_inputs(x, W, b)
    br = run_bass_kernel_spmd(nc, in_maps, list(range(NCORES)),
                              **_cache.get("run_kwargs", {}))
    _cache["last_run"] = br
    oc = np.stack([np.asarray(br.results[i]["oc"]).astype(np.float32)
                   for i in range(NCORES)])
    # oc: [core*pair, (j,o), (k2, g, b)]
    oc = oc.reshape(NCORES * RPC // 2, 4, O, 2, NG, B)
    # out[b, o, row=pair*2+k2, x=g*4+j]
    oc = oc.transpose(5, 2, 0, 3, 4, 1)  # [b, o, pair, k2, g, j]
    out = oc.reshape(B, O, NCORES * RPC // 2 * 2, NG * 4)
    return np.ascontiguousarray(out[:, :, :RY, :])


# revision 71
# speedup vs baseline: 1.0801x; 1.0566x over previous
"""Locally-connected 2D layer on 8 Trainium2 NeuronCores.

Problem: x[128,3,64,64] f32, per-position weights W[60,60,32,75], bias b[60,60,32]
  out[b,o,y,x] = sum_k patches[b,y,x,k] * W[y,x,o,k] + b[y,x,o],  k=(c,dy,dx)

Strategy (spatial sharding over output rows, 8 rows/core, memory-regime):
  - Input ring of 4 BLOCKS x 32 partitions; each block holds TWO input rows
    (15 dx-expanded patch planes + a ones plane that carries the bias, x2).
    Input row r lives at block (r//2)%4, sub-slot r%2.  A block's re-fill
    comes 3+ output rows after its last reader, so fills stream at full
    queue rate instead of serializing with the matmuls (the old mod-5 ring
    stalled ~3.6us per row on the fill chain).
  - Output row k contracts blocks (k//2 .. k//2+2)%4 = 96 partitions (the
    6th covered input row gets zero weights).  Matmul partition bases stay
    32-aligned: unwrapped windows use one matmul, wrapped ones use two
    accumulating matmuls over the low/high partition segments; the host
    compact W layout concatenates active blocks in ascending order.
  - All tensors bf16 on the wire (PSUM f32; host widens output).
  - Queue discipline: hardware DGE queues (sync + scalar) carry x fills,
    W rows 0-3, and the stores (split across both); W rows 4-7 prefetch on
    the gpsimd software queue.  Packets are shaped to >=7680B.
  - Per output row: 5 column chunks x 12 matmul-groups -> psum[32j:32j+32,
    128], DVE copy psum->bf16 pair tile, one 983KB store per row pair.
"""

import numpy as np

B, C, H, WIDTH = 128, 3, 64, 64
KH = KW = 5
RY = RX = 60
O = 32
K = 75
NCORES = 8
RPC = 8             # output rows computed per core (8*8=64, last 4 dropped)
INR = RPC + KH - 1  # 12 input rows per core
PADH = NCORES * RPC + KH - 1  # 68
NG = 15             # groups of 4 x-positions per row
NCH = 5             # matmul column chunks per row (3 groups each)
GPC = 3             # groups per chunk
FXB = RX * B        # 7680 elements per patch plane
NPL = KH * C        # 15 patch planes per input row
SUB = 16            # partitions per input row (15 planes + ones)
BLK = 32            # partitions per block (2 input rows)
NBLK = 4
K2 = 3 * BLK        # contraction depth per output row (96)
WROW = RX * O       # 1920 weight elems per (row, partition)
NWP = 4             # W pieces (2 rows each)

_cache = {}


def _build():
    import concourse.bass as bass
    import concourse.bacc as bacc
    import concourse.tile as tile
    import concourse.mybir as mybir

    f32 = mybir.dt.float32
    bf16 = mybir.dt.bfloat16
    nc = bacc.Bacc("TRN2", target_bir_lowering=False, debug=False,
                   num_devices=NCORES)
    xpr_d = nc.dram_tensor("xpr", [INR, NPL, FXB], bf16, kind="ExternalInput")
    wh_d = nc.dram_tensor("wh", [RPC, 5 * SUB, WROW], bf16,
                          kind="ExternalInput")
    ones_d = nc.dram_tensor("ones", [NBLK * 2, FXB], bf16,
                            kind="ExternalInput")
    oc_d = nc.dram_tensor("oc", [RPC // 2, 128, 2 * NG * B], bf16,
                          kind="ExternalOutput")

    HFB = FXB // 2  # column half (3840 elems -> 7680B packets)
    QFB = FXB // 4  # column quarter (1920 elems)

    with tile.TileContext(nc) as tc:
        with (
            tc.tile_pool(name="const", bufs=1) as cpool,
            tc.tile_pool(name="w", bufs=8) as wpool,
            tc.tile_pool(name="os", bufs=3) as opool,
            tc.tile_pool(name="ps", bufs=6, space=bass.MemorySpace.PSUM) as ppool,
        ):
            xp = cpool.tile([NBLK * BLK, FXB], bf16)  # [128, 7680]

            wts = {}
            # per-row aligned memset covers for the read-but-unloaded
            # partitions (DVE requires 32-aligned bases / 0-64 for 64-spans)
            ZCOV = {0: [(64, 96)], 1: [(0, 32)], 2: [(0, 32), (96, 128)],
                    3: [(0, 64)], 4: [(0, 64)], 5: [(32, 64), (64, 96)],
                    6: [(32, 64), (64, 96)], 7: [(64, 128)]}

            def load_w(k, eng, eng2=None):
                # slot-absolute W tile: zero the inactive window partitions,
                # then DMA the 5 active sub-slots from the compact DRAM row
                # (per-queue column halves when eng2 is given)
                wt = wpool.tile([NBLK * BLK, WROW], bf16, name=f"wp{k}",
                                tag=f"wp{k}", bufs=1)
                for z0, z1 in ZCOV[k]:
                    nc.vector.memset(wt[z0:z1, :], 0.0)
                if k <= 3:
                    # one contiguous run [16k : 16k+80]
                    p0 = SUB * k
                    eng.dma_start(wt[p0:p0 + 5 * SUB, :], wh_d[k])
                else:
                    # wrap: low run on eng, high run on eng2
                    nlo = SUB * (k - 3)
                    eng.dma_start(wt[0:nlo, :], wh_d[k][0:nlo])
                    (eng2 or eng).dma_start(wt[SUB * k:128, :],
                                            wh_d[k][nlo:5 * SUB])
                wts[k] = wt

            def fill(r, f0, f1, eng):
                # input row r -> block (r//2)%4, sub-slot r%2
                p0 = BLK * ((r // 2) % NBLK) + SUB * (r % 2)
                eng.dma_start(xp[p0:p0 + NPL, f0:f1], xpr_d[r, :, f0:f1])

            # ones planes (bias carriers), one per sub-slot; they gate row
            # 0, so keep them off the slow software queue
            for t in range(NBLK):
                for s2 in range(2):
                    p = BLK * t + SUB * s2 + NPL
                    eng = nc.sync if (t + s2) % 2 == 0 else nc.scalar
                    eng.dma_start(xp[p:p + 1, :], ones_d[0:1])
            # MINIMAL prologue: the first matmul waits for every DMA issued
            # before it (compute-phase barrier), so only row 0's true needs
            # go here: ones planes, W row 0, input rows 0-5.  Everything
            # else is issued inside the row loop, after compute starts.
            load_w(0, nc.sync, nc.scalar)
            for r in range(6):
                fill(r, 0, HFB, nc.sync if r % 2 == 0 else nc.scalar)
                fill(r, HFB, FXB, nc.scalar if r % 2 == 0 else nc.sync)

            for k in range(RPC):
                wrow = wts[k]  # [128, 1920], slot-absolute
                m4 = (k // 2) % NBLK
                # W is slot-absolute with zeros on inactive sub-slots, so a
                # full-depth contraction is exact and keeps the PE tile
                # position fixed (pipelined).  m4=0 rows use the legal
                # narrow [0:96) span, which avoids false RAW deps on the
                # block-0/1 refills; other bases would violate the PE's
                # partition-alignment rules, so they contract all 128.
                segs = [(0, 96)] if m4 == 0 else [(0, 128)]
                if k % 2 == 0:
                    ot = opool.tile([128, 2 * NG * B], bf16)  # [128, 3840]
                ob = (k % 2) * NG * B
                for ci in range(NCH):
                    # full 2KB PSUM bank; only the first GPC*B cols are used
                    pt = ppool.tile([128, 4 * B], f32)
                    for gs in range(GPC):
                        for j in range(4):
                            xpos = (ci * GPC + gs) * 4 + j
                            po = pt[32 * j:32 * (j + 1),
                                    gs * B:(gs + 1) * B]
                            wcol = slice(xpos * O, (xpos + 1) * O)
                            xcol = slice(xpos * B, (xpos + 1) * B)
                            for si, (x0, x1) in enumerate(segs):
                                nc.tensor.matmul(
                                    po, wrow[x0:x1, wcol],
                                    xp[x0:x1, xcol],
                                    start=(si == 0),
                                    stop=(si == len(segs) - 1),
                                    tile_position=(x0, 32 * j),
                                )
                    nc.vector.tensor_copy(
                        ot[:, ob + ci * GPC * B:ob + (ci + 1) * GPC * B],
                        pt[:, :GPC * B])
                    if k == 0 and ci == 0:
                        # deferred loads, issued after row 0's first chunk
                        # so they sit past the compute-phase barrier: W row
                        # 1, input rows 6-7, SW-queue W prefetches
                        load_w(1, nc.scalar, nc.sync)
                        fill(6, 0, HFB, nc.sync)
                        fill(6, HFB, FXB, nc.scalar)
                        fill(7, 0, HFB, nc.scalar)
                        fill(7, HFB, FXB, nc.sync)
                        load_w(2, nc.gpsimd)
                        load_w(3, nc.gpsimd)
                        load_w(4, nc.gpsimd)
                    if k in (1, 3) and ci >= 1:
                        # re-fill block (k-1)/2 with input rows k+7, k+8 in
                        # column quarters, each gated on the last chunk of
                        # this row that read those columns
                        f0, f1 = (ci - 1) * QFB, ci * QFB
                        fill(k + 7, f0, f1,
                             nc.sync if ci % 2 == 1 else nc.scalar)
                        fill(k + 8, f0, f1,
                             nc.scalar if ci % 2 == 1 else nc.sync)
                if k == 1:
                    load_w(5, nc.sync, nc.scalar)
                elif k == 3:
                    load_w(6, nc.scalar, nc.sync)
                    load_w(7, nc.sync, nc.scalar)
                if k in (1, 3, 5):
                    nc.sync.dma_start(oc_d[k // 2, 0:64], ot[0:64, :])
                    nc.scalar.dma_start(oc_d[k // 2, 64:128], ot[64:128, :])
                elif k >= 6:
                    # last pair: store each row as soon as it is done, and
                    # split across both queues, to shrink the drain tail
                    c0, c1 = (k % 2) * NG * B, (k % 2 + 1) * NG * B
                    nc.sync.dma_start(oc_d[3, 0:64, c0:c1],
                                      ot[0:64, c0:c1])
                    nc.scalar.dma_start(oc_d[3, 64:128, c0:c1],
                                        ot[64:128, c0:c1])

    nc.compile()
    return nc


def _get_nc():
    if "nc" not in _cache:
        _cache["nc"] = _build()
    return _cache["nc"]


def _prep_inputs(x, W, b):
    import ml_dtypes
    bf = ml_dtypes.bfloat16
    x = np.asarray(x, np.float32)
    W = np.asarray(W, np.float32)
    b = np.asarray(b, np.float32)
    xh = np.zeros((PADH, C, WIDTH, B), np.float32)
    xh[:H] = x.transpose(2, 1, 3, 0)  # [row, c, w, batch]
    # patch planes: xpr_full[r, c*KW+dx, x, b] = xh[r, c, x+dx, b]
    xpr_full = np.zeros((PADH, C, KW, RX, B), np.float32)
    for dx in range(KW):
        xpr_full[:, :, dx] = xh[:, :, dx:dx + RX]
    xpr_full = xpr_full.reshape(PADH, C * KW, FXB)
    Wfull = W.transpose(0, 3, 1, 2)  # [RY, K, RX, O]
    in_maps = []
    for i in range(NCORES):
        # compact W: per row, the 5 active sub-slots (15 weight planes +
        # bias/zero plane each) ordered by ascending absolute partition
        whc = np.zeros((RPC, 5 * SUB, RX, O), np.float32)
        for k in range(RPC):
            y = RPC * i + k
            if y < RY:
                w5 = Wfull[y].reshape(C, KH, KW, RX, O)
                rows = sorted(range(k, k + 5), key=lambda rr: rr % 8)
                for i2, rr in enumerate(rows):
                    dy = rr - k
                    whc[k, i2 * SUB:i2 * SUB + NPL] = \
                        w5[:, dy].reshape(NPL, RX, O)
                    if dy == 0:
                        whc[k, i2 * SUB + NPL] = b[y]
        wres = np.ascontiguousarray(whc.reshape(RPC, 5 * SUB, WROW))
        in_maps.append({
            "xpr": np.ascontiguousarray(
                xpr_full[RPC * i:RPC * i + INR]).astype(bf),
            "wh": wres.astype(bf),
            "ones": np.ones((NBLK * 2, FXB), bf),
        })
    return in_maps


def kernel(x, W, b):
    from concourse.bass_utils import run_bass_kernel_spmd

    nc = _get_nc()
    in_maps = _prep_inputs(x, W, b)
    br = run_bass_kernel_spmd(nc, in_maps, list(range(NCORES)),
                              **_cache.get("run_kwargs", {}))
    _cache["last_run"] = br
    oc = np.stack([np.asarray(br.results[i]["oc"]).astype(np.float32)
                   for i in range(NCORES)])
    # oc: [core*pair, (j,o), (k2, g, b)]
    oc = oc.reshape(NCORES * RPC // 2, 4, O, 2, NG, B)
    # out[b, o, row=pair*2+k2, x=g*4+j]
    oc = oc.transpose(5, 2, 0, 3, 4, 1)  # [b, o, pair, k2, g, j]
    out = oc.reshape(B, O, NCORES * RPC // 2 * 2, NG * 4)
    return np.ascontiguousarray(out[:, :, :RY, :])
